# revision 25
# baseline (speedup 1.0000x reference)
"""BiasedMHA Trainium2 kernel: B=8 batches data-parallel across 8 NeuronCores.

Per core (one batch): fused attention with additive bias + boolean mask.
  out = softmax(Q@K^T*scale + bias, mask) @ V @ Wo^T + bo

v4 design — transposed scores (k on partitions), no DMA transpose:
- host supplies biasT[k,h,q], maskT[k,q], ndT, W.T so every device tensor is
  already in its matmul-native layout; all value math stays on device
- mask folds into the resident biasT tiles once per chunk (DVE bf16 adds
  during the DMA-bound load phase); k=0 stays unmasked by skipping
  partition 0 of chunk 0
- scores^T[k,q] per (kc,h): one K=32 row-banded QK matmul + one I@bias
  full-array matmul accumulate into PSUM; ACT exp evacuates PSUM->SBUF bf16
- attn@V and the softmax denominator both consume e as plain matmuls
  (lhsT = V-slice / ones32, col-banded per head); den is 32-row replicated
  so one reciprocal + one DVE multiply normalizes a whole 4-head pass
- two passes of 4 heads each keep PSUM at exactly 8 banks:
  2x psa (double-buffered) + den + attnV accumulator
"""

import sys

import numpy as np

for _p in ("/opt/trn_rl_repo",):
    if _p not in sys.path:
        sys.path.insert(0, _p)

import concourse.bass as bass  # noqa: E402
import concourse.mybir as mybir  # noqa: E402
import concourse.tile as tile  # noqa: E402
from concourse import bacc  # noqa: E402
from concourse.masks import make_identity  # noqa: E402

NN = 1024  # sequence length
F = 256  # feature dim
H = 8  # heads
D = F // H  # head dim = 32
P = 128  # partitions
KC = NN // P  # 8 k-chunks (also seq tiles)
SCALE = D**-0.5
NEG = -1.0e30

F32 = mybir.dt.float32
BF16 = mybir.dt.bfloat16
U8 = mybir.dt.uint8
AF = mybir.ActivationFunctionType

USE_RECIP_APPROX = True


def build_program():
    """Build the single-core program (one batch). Returns compiled Bacc."""
    nc = bacc.Bacc(
        "TRN2", target_bir_lowering=False, debug=False, num_devices=8
    )

    ndT_dram = nc.dram_tensor("ndT", (F, NN), BF16, kind="ExternalInput").ap()
    biasT_dram = nc.dram_tensor(
        "biasT", (NN, H, NN), BF16, kind="ExternalInput"
    ).ap()
    maskT_dram = nc.dram_tensor(
        "maskT", (NN, NN), U8, kind="ExternalInput"
    ).ap()
    w_dram = {}
    b_dram = {}
    for w in ("q", "k", "v", "o"):
        w_dram[w] = nc.dram_tensor(
            f"wT{w}", (F, F), BF16, kind="ExternalInput"
        ).ap()
        b_dram[w] = nc.dram_tensor(f"b{w}", (F,), F32, kind="ExternalInput").ap()
    out_dram = nc.dram_tensor("out", (NN, F), BF16, kind="ExternalOutput").ap()

    with tile.TileContext(nc) as tc:
        _emit(nc, tc, ndT_dram, biasT_dram, maskT_dram, w_dram, b_dram, out_dram)

    nc.compile()
    return nc


def _emit(nc, tc, ndT_dram, biasT_dram, maskT_dram, w_dram, b_dram, out_dram):
    from contextlib import ExitStack

    ctx = ExitStack()
    with ctx:
        const = ctx.enter_context(tc.tile_pool(name="const", bufs=1))
        biasp = ctx.enter_context(tc.tile_pool(name="biasp", bufs=3))
        biasp1 = ctx.enter_context(tc.tile_pool(name="biasp1", bufs=8))
        mpool = ctx.enter_context(tc.tile_pool(name="mpool", bufs=8))
        qkvp = ctx.enter_context(tc.tile_pool(name="qkvp", bufs=1))
        epool = ctx.enter_context(tc.tile_pool(name="epool", bufs=8))
        spool = ctx.enter_context(tc.tile_pool(name="spool", bufs=4))
        otp = ctx.enter_context(tc.tile_pool(name="otp", bufs=2))
        yp = ctx.enter_context(tc.tile_pool(name="yp", bufs=2))
        psA = ctx.enter_context(tc.tile_pool(name="psA", bufs=2, space="PSUM"))
        psD = ctx.enter_context(tc.tile_pool(name="psD", bufs=1, space="PSUM"))
        psC = ctx.enter_context(tc.tile_pool(name="psC", bufs=1, space="PSUM"))

        # ---- constants ----
        Ibf = const.tile([P, P], BF16, tag="Ibf")
        make_identity(nc, Ibf)
        ones32 = const.tile([P, D], BF16, tag="ones32")
        nc.vector.memset(ones32, 1.0)
        ones1 = const.tile([1, P], BF16, tag="ones1")
        nc.vector.memset(ones1, 1.0)

        wsb = {}
        for w in ("q", "k", "v", "o"):
            wt = const.tile([P, 2, F], BF16, tag=f"w{w}", name=f"w{w}sb")
            nc.sync.dma_start(
                out=wt, in_=w_dram[w].rearrange("(c p) f -> p c f", p=P)
            )
            wsb[w] = wt
        nT = const.tile([P, 2, NN], BF16, tag="nT")
        nc.sync.dma_start(out=nT, in_=ndT_dram.rearrange("(c p) q -> p c q", p=P))

        bqc = const.tile([P, 2], F32, tag="bqc")
        nc.sync.dma_start(out=bqc, in_=b_dram["q"].rearrange("(c p) -> p c", p=P))
        bqs = const.tile([P, 2], F32, tag="bqs")
        nc.vector.tensor_scalar_mul(bqs, bqc, SCALE)
        bkc = const.tile([P, 2], F32, tag="bkc")
        nc.sync.dma_start(out=bkc, in_=b_dram["k"].rearrange("(c p) -> p c", p=P))
        brow = {}
        for w in ("v", "o"):
            bf = const.tile([1, F], F32, tag=f"b{w}f", name=f"b{w}f")
            nc.sync.dma_start(out=bf, in_=b_dram[w][None, :])
            bh = const.tile([1, F], BF16, tag=f"b{w}h", name=f"b{w}h")
            nc.vector.tensor_copy(bh, bf)
            brow[w] = bh

        # ---- prologue: Q/K/V projections ----
        # qt/kt[p, hg, q]: head hg*4+j lives at partitions 32j..32j+31
        qt = qkvp.tile([P, 2, NN], BF16, tag="qt")
        kt = qkvp.tile([P, 2, NN], BF16, tag="kt")
        for name, dst in (("q", qt), ("k", kt)):
            for co in range(2):
                ps = psA.tile([P, NN], F32, tag="A", name=f"ps_{name}{co}")
                for s in range(2):
                    sl = slice(s * 512, (s + 1) * 512)
                    for ci in range(2):
                        nc.tensor.matmul(
                            ps[:, sl],
                            lhsT=wsb[name][:, ci, co * P : (co + 1) * P],
                            rhs=nT[:, ci, sl],
                            start=(ci == 0),
                            stop=(ci == 1),
                        )
                if name == "q":
                    nc.scalar.activation(
                        dst[:, co, :], ps, AF.Identity,
                        bias=bqs[:, co : co + 1], scale=SCALE,
                    )
                else:
                    nc.scalar.activation(
                        dst[:, co, :], ps, AF.Identity, bias=bkc[:, co : co + 1]
                    )

        # vp[p, kc, (h d)]: V rows for k-chunk kc
        vp = qkvp.tile([P, KC, F], BF16, tag="vp")
        for t in range(KC):
            psv = psA.tile([P, F], F32, tag="A", name=f"psv{t}")
            for ci in range(2):
                nc.tensor.matmul(
                    psv,
                    lhsT=nT[:, ci, t * P : (t + 1) * P],
                    rhs=wsb["v"][:, ci, :],
                    start=(ci == 0),
                    stop=False,
                )
            nc.tensor.matmul(psv, lhsT=ones1, rhs=brow["v"], start=False, stop=True)
            nc.scalar.copy(vp[:, t, :], psv)

        # ---- load phase ----
        # The bias streams in per-pass halves ([k, 0:4, q] then [k, 4:8, q]):
        # same total HBM traffic, but pass0 starts after ~1MB instead of
        # waiting out a 17MB resident load, and SBUF holds at most a few
        # half-chunks. The DMA FIFO order (pass0 halves, then pass1 halves)
        # makes pass1's bias arrive during pass0's compute.
        def bias_half_dma(hg, kc, first=False):
            bt = bias_pools[hg].tile(
                [P, 4 * NN], BF16, tag=f"bias{hg}", name=f"biasT_{hg}_{kc}"
            )
            nc.sync.dma_start(
                out=bt,
                in_=biasT_dram[
                    kc * P : (kc + 1) * P, 4 * hg : 4 * (hg + 1)
                ].rearrange("k h q -> k (h q)"),
            )
            bias_t[hg].append(bt)

        bias_pools = {0: biasp, 1: biasp1}
        bias_t = {0: [], 1: []}
        bias_half_dma(0, 0)
        m8 = []
        for kc in range(KC):
            m = mpool.tile([P, NN], U8, tag="m8", name=f"m8_{kc}")
            nc.sync.dma_start(out=m, in_=maskT_dram[kc * P : (kc + 1) * P, :])
            m8.append(m)
        for kc in range(1, KC):
            bias_half_dma(0, kc)
        for kc in range(KC):
            bias_half_dma(1, kc)
        # mask -> -1e30 bf16, then fold into each head's bias slice (DVE;
        # pass1 folds queue behind pass0's and behind their own DMAs, so
        # they run in the pass0 window where the DVE is otherwise idle).
        # chunk 0 partition 0 is the always-unmasked k=0 row: zero its mask.
        mnegs = []
        for kc in range(KC):
            mneg = mpool.tile([P, NN], BF16, tag="mneg", name=f"mneg_{kc}", bufs=8)
            nc.scalar.mul(mneg, m8[kc], NEG)
            if kc == 0:
                nc.gpsimd.memset(mneg[0:1, :], 0.0)
            mnegs.append(mneg)
        for hg in range(2):
            for kc in range(KC):
                bv = bias_t[hg][kc].rearrange("k (h q) -> k h q", q=NN)
                bvb, mbb = bass.broadcast_tensor_aps(bv, mnegs[kc][:, None, :])
                nc.vector.tensor_add(bvb, bvb, mbb)

        # ---- two passes of 4 heads each ----
        outT = []
        for hg in range(2):
            den = psD.tile([P, NN], F32, tag="D", name=f"den{hg}")
            psc = psC.tile([P, NN], F32, tag="C", name=f"psc{hg}")

            def emit_backs(kc, etiles):
                # kind-grouped across the 4 col bands: consecutive same-kind
                # M=32 matmuls to distinct 32-col array bands can pack, and
                # the den matmuls share the ones32 stationary operand.
                # start=True per band at kc==0 clears/overwrites only that
                # matmul's own PSUM footprint (per-element has_written).
                for s in range(2):
                    sl = slice(s * 512, (s + 1) * 512)
                    for j in range(4):
                        nc.tensor.matmul(
                            den[32 * j : 32 * (j + 1), sl],
                            lhsT=ones32,
                            rhs=etiles[j][:, sl],
                            start=(kc == 0),
                            stop=(kc == KC - 1),
                            tile_position=(0, 32 * j),
                            skip_group_check=True,
                        )
                for s in range(2):
                    sl = slice(s * 512, (s + 1) * 512)
                    for j in range(4):
                        h = hg * 4 + j
                        nc.tensor.matmul(
                            psc[32 * j : 32 * (j + 1), sl],
                            lhsT=vp[:, kc, h * D : (h + 1) * D],
                            rhs=etiles[j][:, sl],
                            start=(kc == 0),
                            stop=(kc == KC - 1),
                            tile_position=(0, 32 * j),
                            skip_group_check=True,
                        )

            prev = None
            for kc in range(KC):
                cur = []
                for j in range(4):
                    h = hg * 4 + j
                    psa = psA.tile([P, NN], F32, tag="A", name=f"psa_{hg}_{kc}_{j}")
                    # both halves of QK share one kt LDWEIGHTS; on-PE bias
                    # adds share one Ibf LDWEIGHTS
                    on_pe = hg == 0 and j < 2
                    for s in range(2):
                        sl = slice(s * 512, (s + 1) * 512)
                        nc.tensor.matmul(
                            psa[:, sl],
                            lhsT=kt[32 * j : 32 * (j + 1), hg, kc * P : (kc + 1) * P],
                            rhs=qt[32 * j : 32 * (j + 1), hg, sl],
                            start=True,
                            stop=not on_pe,
                            tile_position=(32 * j, 0),
                            skip_group_check=True,
                        )
                    e = epool.tile([P, NN], BF16, tag="e", name=f"e_{hg}_{kc}_{j}")
                    if on_pe:
                        # bias add on PE while the load phase gates the pace
                        for s in range(2):
                            sl = slice(s * 512, (s + 1) * 512)
                            nc.tensor.matmul(
                                psa[:, sl],
                                lhsT=Ibf,
                                rhs=bias_t[0][kc][
                                    :, j * NN + s * 512 : j * NN + (s + 1) * 512
                                ],
                                start=False,
                                stop=True,
                                skip_group_check=True,
                            )
                        nc.scalar.activation(e, psa, AF.Exp)
                    else:
                        # bias add on DVE
                        sP = spool.tile([P, NN], BF16, tag="sP", name=f"sP_{hg}_{kc}_{j}")
                        nc.vector.tensor_add(
                            sP, psa, bias_t[hg][kc][:, j * NN : (j + 1) * NN]
                        )
                        nc.scalar.activation(e, sP, AF.Exp)
                    cur.append(e)
                if prev is not None:
                    emit_backs(kc - 1, prev)
                prev = cur
            emit_backs(KC - 1, prev)

            rec = yp.tile([P, NN], F32, tag="rec", name=f"rec{hg}", bufs=1)
            if USE_RECIP_APPROX:
                nc.vector.reciprocal_approx_fast(rec, den)
            else:
                nc.vector.reciprocal(rec, den)
            oT = otp.tile([P, NN], BF16, tag="oT", name=f"outT{hg}")
            nc.vector.tensor_mul(oT, psc, rec)
            outT.append(oT)

        # ---- output projection (psy spread over all psum pools so the
        # eight t-chunks overlap instead of serializing on two slots) ----
        ypool = [(psA, "A"), (psA, "A"), (psD, "D"), (psC, "C")]
        for t in range(KC):
            pool, ptag = ypool[t % 4]
            psy = pool.tile([P, F], F32, tag=ptag, name=f"psy{t}")
            for hg in range(2):
                nc.tensor.matmul(
                    psy,
                    lhsT=outT[hg][:, t * P : (t + 1) * P],
                    rhs=wsb["o"][:, hg, :],
                    start=(hg == 0),
                    stop=False,
                )
            nc.tensor.matmul(psy, lhsT=ones1, rhs=brow["o"], start=False, stop=True)
            y = yp.tile([P, F], BF16, tag="y", name=f"y{t}")
            nc.scalar.copy(y, psy)
            nc.sync.dma_start(out=out_dram[t * P : (t + 1) * P, :], in_=y)


_CACHE = {}


def _make_in_maps(inputs):
    import ml_dtypes

    bf16 = ml_dtypes.bfloat16
    nd = np.asarray(inputs["ndata"], np.float32)
    ab = np.asarray(inputs["attn_bias"], np.float32).astype(bf16)
    am = np.asarray(inputs["attn_mask"]).astype(np.uint8)
    ws = {
        f"wT{w}": np.ascontiguousarray(
            np.asarray(inputs[f"W{w}"], np.float32).T
        ).astype(bf16)
        for w in ("q", "k", "v", "o")
    }
    bs = {
        f"b{w}": np.asarray(inputs[f"b{w}"], np.float32) for w in ("q", "k", "v", "o")
    }
    in_maps = []
    for b in range(nd.shape[0]):
        m = {
            "ndT": np.ascontiguousarray(nd[b].T).astype(bf16),
            "biasT": np.ascontiguousarray(ab[b].transpose(1, 2, 0)),
            "maskT": np.ascontiguousarray(am[b].T),
        }
        m.update(ws)
        m.update(bs)
        in_maps.append(m)
    return in_maps


def _get_nc():
    if "nc" not in _CACHE:
        _CACHE["nc"] = build_program()
    return _CACHE["nc"]


def _ensure_ntff_hook():
    """Shim antenv.axon_hooks (absent in this image) so trace=True works."""
    import types

    try:
        from antenv.axon_hooks import get_axon_ntff_profile_hook  # noqa: F401

        return
    except ImportError:
        pass
    import antenv

    mod = types.ModuleType("antenv.axon_hooks")
    _h = [None]
    mod.set_axon_ntff_profile_hook = lambda h: _h.__setitem__(0, h)
    mod.get_axon_ntff_profile_hook = lambda: _h[0]
    sys.modules["antenv.axon_hooks"] = mod
    antenv.axon_hooks = mod
    from trn_agent_boot.trn_boot import _ntff_profile_via_ctypes

    mod.set_axon_ntff_profile_hook(
        _ntff_profile_via_ctypes("/opt/axon/libaxon_pjrt.so")
    )


def run(inputs, trace=False):
    """Run on hardware; returns (output (B,N,F) f32, exec_time_ns or None)."""
    from concourse import bass_utils

    if trace:
        _ensure_ntff_hook()
    nc = _get_nc()
    in_maps = _make_in_maps(inputs)
    res = bass_utils.run_bass_kernel_spmd(
        nc, in_maps, core_ids=list(range(len(in_maps))), trace=trace
    )
    out = np.stack([r["out"] for r in res.results]).astype(np.float32)
    return out, res.exec_time_ns


def kernel(**inputs):
    out, _ = run(inputs, trace=False)
    return out


# revision 27
# speedup vs baseline: 1.0416x; 1.0416x over previous
"""BiasedMHA Trainium2 kernel: B=8 batches data-parallel across 8 NeuronCores.

Per core (one batch): fused attention with additive bias + boolean mask.
  out = softmax(Q@K^T*scale + bias, mask) @ V @ Wo^T + bo

v4 design — transposed scores (k on partitions), no DMA transpose:
- host supplies biasT[k,h,q], maskT[k,q], ndT, W.T so every device tensor is
  already in its matmul-native layout; all value math stays on device
- mask folds into the resident biasT tiles once per chunk (DVE bf16 adds
  during the DMA-bound load phase); k=0 stays unmasked by skipping
  partition 0 of chunk 0
- scores^T[k,q] per (kc,h): one K=32 row-banded QK matmul + one I@bias
  full-array matmul accumulate into PSUM; ACT exp evacuates PSUM->SBUF bf16
- attn@V and the softmax denominator both consume e as plain matmuls
  (lhsT = V-slice / ones32, col-banded per head); den is 32-row replicated
  so one reciprocal + one DVE multiply normalizes a whole 4-head pass
- two passes of 4 heads each keep PSUM at exactly 8 banks:
  2x psa (double-buffered) + den + attnV accumulator
"""

import sys

import numpy as np

for _p in ("/opt/trn_rl_repo",):
    if _p not in sys.path:
        sys.path.insert(0, _p)

import concourse.bass as bass  # noqa: E402
import concourse.mybir as mybir  # noqa: E402
import concourse.tile as tile  # noqa: E402
from concourse import bacc  # noqa: E402
from concourse.masks import make_identity  # noqa: E402

NN = 1024  # sequence length
F = 256  # feature dim
H = 8  # heads
D = F // H  # head dim = 32
P = 128  # partitions
KC = NN // P  # 8 k-chunks (also seq tiles)
SCALE = D**-0.5
NEG = -1.0e30

F32 = mybir.dt.float32
BF16 = mybir.dt.bfloat16
U8 = mybir.dt.uint8
AF = mybir.ActivationFunctionType

USE_RECIP_APPROX = True


def build_program():
    """Build the single-core program (one batch). Returns compiled Bacc."""
    nc = bacc.Bacc(
        "TRN2", target_bir_lowering=False, debug=False, num_devices=8
    )

    ndT_dram = nc.dram_tensor("ndT", (F, NN), BF16, kind="ExternalInput").ap()
    biasT_dram = nc.dram_tensor(
        "biasT", (NN, H, NN), BF16, kind="ExternalInput"
    ).ap()
    maskT_dram = nc.dram_tensor(
        "maskT", (NN, NN), U8, kind="ExternalInput"
    ).ap()
    w_dram = {}
    b_dram = {}
    for w in ("q", "k", "v", "o"):
        w_dram[w] = nc.dram_tensor(
            f"wT{w}", (F, F), BF16, kind="ExternalInput"
        ).ap()
        b_dram[w] = nc.dram_tensor(f"b{w}", (F,), F32, kind="ExternalInput").ap()
    out_dram = nc.dram_tensor("out", (NN, F), BF16, kind="ExternalOutput").ap()

    with tile.TileContext(nc) as tc:
        _emit(nc, tc, ndT_dram, biasT_dram, maskT_dram, w_dram, b_dram, out_dram)

    nc.compile()
    return nc


def _emit(nc, tc, ndT_dram, biasT_dram, maskT_dram, w_dram, b_dram, out_dram):
    from contextlib import ExitStack

    ctx = ExitStack()
    with ctx:
        const = ctx.enter_context(tc.tile_pool(name="const", bufs=1))
        biasp = ctx.enter_context(tc.tile_pool(name="biasp", bufs=3))
        biasp1 = ctx.enter_context(tc.tile_pool(name="biasp1", bufs=8))
        mpool = ctx.enter_context(tc.tile_pool(name="mpool", bufs=8))
        qkvp = ctx.enter_context(tc.tile_pool(name="qkvp", bufs=1))
        epool = ctx.enter_context(tc.tile_pool(name="epool", bufs=8))
        spool = ctx.enter_context(tc.tile_pool(name="spool", bufs=4))
        otp = ctx.enter_context(tc.tile_pool(name="otp", bufs=2))
        yp = ctx.enter_context(tc.tile_pool(name="yp", bufs=2))
        psA = ctx.enter_context(tc.tile_pool(name="psA", bufs=2, space="PSUM"))
        psD = ctx.enter_context(tc.tile_pool(name="psD", bufs=1, space="PSUM"))
        psC = ctx.enter_context(tc.tile_pool(name="psC", bufs=1, space="PSUM"))

        # ---- constants ----
        Ibf = const.tile([P, P], BF16, tag="Ibf")
        make_identity(nc, Ibf)
        ones32 = const.tile([P, D], BF16, tag="ones32")
        nc.vector.memset(ones32, 1.0)
        ones1 = const.tile([1, P], BF16, tag="ones1")
        nc.vector.memset(ones1, 1.0)

        wsb = {}
        for w in ("q", "k", "v", "o"):
            wt = const.tile([P, 2, F], BF16, tag=f"w{w}", name=f"w{w}sb")
            nc.sync.dma_start(
                out=wt, in_=w_dram[w].rearrange("(c p) f -> p c f", p=P)
            )
            wsb[w] = wt
        nT = const.tile([P, 2, NN], BF16, tag="nT")
        nc.sync.dma_start(out=nT, in_=ndT_dram.rearrange("(c p) q -> p c q", p=P))

        bqc = const.tile([P, 2], F32, tag="bqc")
        nc.sync.dma_start(out=bqc, in_=b_dram["q"].rearrange("(c p) -> p c", p=P))
        bqs = const.tile([P, 2], F32, tag="bqs")
        nc.vector.tensor_scalar_mul(bqs, bqc, SCALE)
        bkc = const.tile([P, 2], F32, tag="bkc")
        nc.sync.dma_start(out=bkc, in_=b_dram["k"].rearrange("(c p) -> p c", p=P))
        brow = {}
        for w in ("v", "o"):
            bf = const.tile([1, F], F32, tag=f"b{w}f", name=f"b{w}f")
            nc.sync.dma_start(out=bf, in_=b_dram[w][None, :])
            bh = const.tile([1, F], BF16, tag=f"b{w}h", name=f"b{w}h")
            nc.vector.tensor_copy(bh, bf)
            brow[w] = bh

        # ---- prologue: Q/K/V projections ----
        # qt/kt[p, hg, q]: head hg*4+j lives at partitions 32j..32j+31
        qt = qkvp.tile([P, 2, NN], BF16, tag="qt")
        kt = qkvp.tile([P, 2, NN], BF16, tag="kt")
        for name, dst in (("q", qt), ("k", kt)):
            for co in range(2):
                ps = psA.tile([P, NN], F32, tag="A", name=f"ps_{name}{co}")
                for s in range(2):
                    sl = slice(s * 512, (s + 1) * 512)
                    for ci in range(2):
                        nc.tensor.matmul(
                            ps[:, sl],
                            lhsT=wsb[name][:, ci, co * P : (co + 1) * P],
                            rhs=nT[:, ci, sl],
                            start=(ci == 0),
                            stop=(ci == 1),
                        )
                if name == "q":
                    nc.scalar.activation(
                        dst[:, co, :], ps, AF.Identity,
                        bias=bqs[:, co : co + 1], scale=SCALE,
                    )
                else:
                    nc.scalar.activation(
                        dst[:, co, :], ps, AF.Identity, bias=bkc[:, co : co + 1]
                    )

        # vp[p, kc, (h d)]: V rows for k-chunk kc
        vp = qkvp.tile([P, KC, F], BF16, tag="vp")
        for t in range(KC):
            psv = psA.tile([P, F], F32, tag="A", name=f"psv{t}")
            for ci in range(2):
                nc.tensor.matmul(
                    psv,
                    lhsT=nT[:, ci, t * P : (t + 1) * P],
                    rhs=wsb["v"][:, ci, :],
                    start=(ci == 0),
                    stop=False,
                )
            nc.tensor.matmul(psv, lhsT=ones1, rhs=brow["v"], start=False, stop=True)
            nc.scalar.copy(vp[:, t, :], psv)

        # ---- load phase ----
        # The bias streams in per-pass halves ([k, 0:4, q] then [k, 4:8, q]):
        # same total HBM traffic, but pass0 starts after ~1MB instead of
        # waiting out a 17MB resident load, and SBUF holds at most a few
        # half-chunks. The DMA FIFO order (pass0 halves, then pass1 halves)
        # makes pass1's bias arrive during pass0's compute.
        def bias_half_dma(hg, kc, first=False):
            bt = bias_pools[hg].tile(
                [P, 4 * NN], BF16, tag=f"bias{hg}", name=f"biasT_{hg}_{kc}"
            )
            nc.sync.dma_start(
                out=bt,
                in_=biasT_dram[
                    kc * P : (kc + 1) * P, 4 * hg : 4 * (hg + 1)
                ].rearrange("k h q -> k (h q)"),
            )
            bias_t[hg].append(bt)

        bias_pools = {0: biasp, 1: biasp1}
        bias_t = {0: [], 1: []}
        bias_half_dma(0, 0)
        m8 = []
        for kc in range(KC):
            m = mpool.tile([P, NN], U8, tag="m8", name=f"m8_{kc}")
            nc.sync.dma_start(out=m, in_=maskT_dram[kc * P : (kc + 1) * P, :])
            m8.append(m)
        for kc in range(1, KC):
            bias_half_dma(0, kc)
        for kc in range(KC):
            bias_half_dma(1, kc)
        # mask -> -1e30 bf16 (ACT), then fold into each head's bias slice
        # with one broadcast DVE add per half-chunk. Emission order matters:
        # each engine queue is FIFO, so pass0's folds interleave into the
        # pass0 loop (keeping the first exps near the queue head) and
        # pass1's folds are emitted between the passes (their DMAs land
        # mid-pass0; queueing them earlier would stall pass0's DVE adds).
        # chunk 0 partition 0 is the always-unmasked k=0 row: zero its mask.
        mnegs = {}

        def fold(hg, kc):
            if kc not in mnegs:
                mneg = mpool.tile(
                    [P, NN], BF16, tag="mneg", name=f"mneg_{kc}", bufs=8
                )
                nc.scalar.mul(mneg, m8[kc], NEG)
                if kc == 0:
                    nc.gpsimd.memset(mneg[0:1, :], 0.0)
                mnegs[kc] = mneg
            bv = bias_t[hg][kc].rearrange("k (h q) -> k h q", q=NN)
            bvb, mbb = bass.broadcast_tensor_aps(bv, mnegs[kc][:, None, :])
            nc.vector.tensor_add(bvb, bvb, mbb)

        fold(0, 0)
        fold(0, 1)

        # ---- two passes of 4 heads each ----
        outT = []
        for hg in range(2):
            if hg == 1:
                for kc in range(KC):
                    fold(1, kc)
            den = psD.tile([P, NN], F32, tag="D", name=f"den{hg}")
            psc = psC.tile([P, NN], F32, tag="C", name=f"psc{hg}")

            def emit_backs(kc, etiles):
                # kind-grouped across the 4 col bands: consecutive same-kind
                # M=32 matmuls to distinct 32-col array bands can pack, and
                # the den matmuls share the ones32 stationary operand.
                # start=True per band at kc==0 clears/overwrites only that
                # matmul's own PSUM footprint (per-element has_written).
                for s in range(2):
                    sl = slice(s * 512, (s + 1) * 512)
                    for j in range(4):
                        nc.tensor.matmul(
                            den[32 * j : 32 * (j + 1), sl],
                            lhsT=ones32,
                            rhs=etiles[j][:, sl],
                            start=(kc == 0),
                            stop=(kc == KC - 1),
                            tile_position=(0, 32 * j),
                            skip_group_check=True,
                        )
                for s in range(2):
                    sl = slice(s * 512, (s + 1) * 512)
                    for j in range(4):
                        h = hg * 4 + j
                        nc.tensor.matmul(
                            psc[32 * j : 32 * (j + 1), sl],
                            lhsT=vp[:, kc, h * D : (h + 1) * D],
                            rhs=etiles[j][:, sl],
                            start=(kc == 0),
                            stop=(kc == KC - 1),
                            tile_position=(0, 32 * j),
                            skip_group_check=True,
                        )

            prev = None
            for kc in range(KC):
                if hg == 0 and kc + 2 < KC:
                    fold(0, kc + 2)
                cur = []
                for j in range(4):
                    h = hg * 4 + j
                    psa = psA.tile([P, NN], F32, tag="A", name=f"psa_{hg}_{kc}_{j}")
                    # both halves of QK share one kt LDWEIGHTS; on-PE bias
                    # adds share one Ibf LDWEIGHTS
                    on_pe = hg == 0 and j < 2
                    for s in range(2):
                        sl = slice(s * 512, (s + 1) * 512)
                        nc.tensor.matmul(
                            psa[:, sl],
                            lhsT=kt[32 * j : 32 * (j + 1), hg, kc * P : (kc + 1) * P],
                            rhs=qt[32 * j : 32 * (j + 1), hg, sl],
                            start=True,
                            stop=not on_pe,
                            tile_position=(32 * j, 0),
                            skip_group_check=True,
                        )
                    e = epool.tile([P, NN], BF16, tag="e", name=f"e_{hg}_{kc}_{j}")
                    if on_pe:
                        # bias add on PE while the load phase gates the pace
                        for s in range(2):
                            sl = slice(s * 512, (s + 1) * 512)
                            nc.tensor.matmul(
                                psa[:, sl],
                                lhsT=Ibf,
                                rhs=bias_t[0][kc][
                                    :, j * NN + s * 512 : j * NN + (s + 1) * 512
                                ],
                                start=False,
                                stop=True,
                                skip_group_check=True,
                            )
                        nc.scalar.activation(e, psa, AF.Exp)
                    else:
                        # bias add on DVE
                        sP = spool.tile([P, NN], BF16, tag="sP", name=f"sP_{hg}_{kc}_{j}")
                        nc.vector.tensor_add(
                            sP, psa, bias_t[hg][kc][:, j * NN : (j + 1) * NN]
                        )
                        nc.scalar.activation(e, sP, AF.Exp)
                    cur.append(e)
                if prev is not None:
                    emit_backs(kc - 1, prev)
                prev = cur
            emit_backs(KC - 1, prev)

            rec = yp.tile([P, NN], F32, tag="rec", name=f"rec{hg}", bufs=1)
            if USE_RECIP_APPROX:
                nc.vector.reciprocal_approx_fast(rec, den)
            else:
                nc.vector.reciprocal(rec, den)
            oT = otp.tile([P, NN], BF16, tag="oT", name=f"outT{hg}")
            nc.vector.tensor_mul(oT, psc, rec)
            outT.append(oT)

        # ---- output projection (psy spread over all psum pools so the
        # eight t-chunks overlap instead of serializing on two slots) ----
        ypool = [(psA, "A"), (psA, "A"), (psD, "D"), (psC, "C")]
        for t in range(KC):
            pool, ptag = ypool[t % 4]
            psy = pool.tile([P, F], F32, tag=ptag, name=f"psy{t}")
            for hg in range(2):
                nc.tensor.matmul(
                    psy,
                    lhsT=outT[hg][:, t * P : (t + 1) * P],
                    rhs=wsb["o"][:, hg, :],
                    start=(hg == 0),
                    stop=False,
                )
            nc.tensor.matmul(psy, lhsT=ones1, rhs=brow["o"], start=False, stop=True)
            y = yp.tile([P, F], BF16, tag="y", name=f"y{t}")
            nc.scalar.copy(y, psy)
            nc.sync.dma_start(out=out_dram[t * P : (t + 1) * P, :], in_=y)


_CACHE = {}


def _make_in_maps(inputs):
    import ml_dtypes

    bf16 = ml_dtypes.bfloat16
    nd = np.asarray(inputs["ndata"], np.float32)
    ab = np.asarray(inputs["attn_bias"], np.float32).astype(bf16)
    am = np.asarray(inputs["attn_mask"]).astype(np.uint8)
    ws = {
        f"wT{w}": np.ascontiguousarray(
            np.asarray(inputs[f"W{w}"], np.float32).T
        ).astype(bf16)
        for w in ("q", "k", "v", "o")
    }
    bs = {
        f"b{w}": np.asarray(inputs[f"b{w}"], np.float32) for w in ("q", "k", "v", "o")
    }
    in_maps = []
    for b in range(nd.shape[0]):
        m = {
            "ndT": np.ascontiguousarray(nd[b].T).astype(bf16),
            "biasT": np.ascontiguousarray(ab[b].transpose(1, 2, 0)),
            "maskT": np.ascontiguousarray(am[b].T),
        }
        m.update(ws)
        m.update(bs)
        in_maps.append(m)
    return in_maps


def _get_nc():
    if "nc" not in _CACHE:
        _CACHE["nc"] = build_program()
    return _CACHE["nc"]


def _ensure_ntff_hook():
    """Shim antenv.axon_hooks (absent in this image) so trace=True works."""
    import types

    try:
        from antenv.axon_hooks import get_axon_ntff_profile_hook  # noqa: F401

        return
    except ImportError:
        pass
    import antenv

    mod = types.ModuleType("antenv.axon_hooks")
    _h = [None]
    mod.set_axon_ntff_profile_hook = lambda h: _h.__setitem__(0, h)
    mod.get_axon_ntff_profile_hook = lambda: _h[0]
    sys.modules["antenv.axon_hooks"] = mod
    antenv.axon_hooks = mod
    from trn_agent_boot.trn_boot import _ntff_profile_via_ctypes

    mod.set_axon_ntff_profile_hook(
        _ntff_profile_via_ctypes("/opt/axon/libaxon_pjrt.so")
    )


def run(inputs, trace=False):
    """Run on hardware; returns (output (B,N,F) f32, exec_time_ns or None)."""
    from concourse import bass_utils

    if trace:
        _ensure_ntff_hook()
    nc = _get_nc()
    in_maps = _make_in_maps(inputs)
    res = bass_utils.run_bass_kernel_spmd(
        nc, in_maps, core_ids=list(range(len(in_maps))), trace=trace
    )
    out = np.stack([r["out"] for r in res.results]).astype(np.float32)
    return out, res.exec_time_ns


def kernel(**inputs):
    out, _ = run(inputs, trace=False)
    return out


# revision 28
# speedup vs baseline: 1.0451x; 1.0034x over previous
"""BiasedMHA Trainium2 kernel: B=8 batches data-parallel across 8 NeuronCores.

Per core (one batch): fused attention with additive bias + boolean mask.
  out = softmax(Q@K^T*scale + bias, mask) @ V @ Wo^T + bo

Design — transposed scores (k on partitions), no DMA transpose:
- host supplies biasT[k,h,q], maskT[k,q], ndT, W.T so every device tensor is
  already in its matmul-native layout; all value math stays on device
- the bias streams from HBM in per-pass 4-head halves (same total traffic,
  but the first pass starts after ~1MB and SBUF never holds the full 17MB);
  the mask folds into each half-chunk with one broadcast DVE add; k=0 stays
  unmasked by zeroing partition 0 of mneg chunk 0
- scores^T[k,q] per (kc,h): K=32 row-banded QK matmul, then the bias lands
  via I@bias full-array matmul (PE, pass0 j<2) or a DVE tensor_add (rest) —
  split to balance the power-throttled PE against the DVE; ACT exp
  evacuates to SBUF bf16
- attn@V and the softmax denominator consume e as plain matmuls (lhsT =
  V-slice / ones32), emitted kind-grouped across the four 32-col bands so
  they pack on the PE array; den is 32-row replicated so one
  reciprocal_approx_fast + one DVE multiply normalizes a whole 4-head pass
- per-band PSUM accumulation interleaves across bands with start=True on
  each band's first chunk (has_written clearing is per-matmul-footprint)
- two passes of 4 heads keep PSUM at exactly 8 banks: 2x psa double-buffered
  + den + attnV accumulator; engine FIFOs are scheduled by emission order
  (pass1's folds emitted between the passes, pass0's interleaved)
"""

import sys

import numpy as np

for _p in ("/opt/trn_rl_repo",):
    if _p not in sys.path:
        sys.path.insert(0, _p)

import concourse.bass as bass  # noqa: E402
import concourse.mybir as mybir  # noqa: E402
import concourse.tile as tile  # noqa: E402
from concourse import bacc  # noqa: E402
from concourse.masks import make_identity  # noqa: E402

NN = 1024  # sequence length
F = 256  # feature dim
H = 8  # heads
D = F // H  # head dim = 32
P = 128  # partitions
KC = NN // P  # 8 k-chunks (also seq tiles)
SCALE = D**-0.5
NEG = -1.0e30

F32 = mybir.dt.float32
BF16 = mybir.dt.bfloat16
U8 = mybir.dt.uint8
AF = mybir.ActivationFunctionType

USE_RECIP_APPROX = True


def build_program():
    """Build the single-core program (one batch). Returns compiled Bacc."""
    nc = bacc.Bacc(
        "TRN2", target_bir_lowering=False, debug=False, num_devices=8
    )

    ndT_dram = nc.dram_tensor("ndT", (F, NN), BF16, kind="ExternalInput").ap()
    biasT_dram = nc.dram_tensor(
        "biasT", (NN, H, NN), BF16, kind="ExternalInput"
    ).ap()
    maskT_dram = nc.dram_tensor(
        "maskT", (NN, NN), U8, kind="ExternalInput"
    ).ap()
    w_dram = {}
    b_dram = {}
    for w in ("q", "k", "v", "o"):
        w_dram[w] = nc.dram_tensor(
            f"wT{w}", (F, F), BF16, kind="ExternalInput"
        ).ap()
        b_dram[w] = nc.dram_tensor(f"b{w}", (F,), F32, kind="ExternalInput").ap()
    out_dram = nc.dram_tensor("out", (NN, F), BF16, kind="ExternalOutput").ap()

    with tile.TileContext(nc) as tc:
        _emit(nc, tc, ndT_dram, biasT_dram, maskT_dram, w_dram, b_dram, out_dram)

    nc.compile()
    return nc


def _emit(nc, tc, ndT_dram, biasT_dram, maskT_dram, w_dram, b_dram, out_dram):
    from contextlib import ExitStack

    ctx = ExitStack()
    with ctx:
        const = ctx.enter_context(tc.tile_pool(name="const", bufs=1))
        biasp = ctx.enter_context(tc.tile_pool(name="biasp", bufs=3))
        biasp1 = ctx.enter_context(tc.tile_pool(name="biasp1", bufs=8))
        mpool = ctx.enter_context(tc.tile_pool(name="mpool", bufs=8))
        qkvp = ctx.enter_context(tc.tile_pool(name="qkvp", bufs=1))
        epool = ctx.enter_context(tc.tile_pool(name="epool", bufs=8))
        spool = ctx.enter_context(tc.tile_pool(name="spool", bufs=4))
        otp = ctx.enter_context(tc.tile_pool(name="otp", bufs=2))
        yp = ctx.enter_context(tc.tile_pool(name="yp", bufs=2))
        psA = ctx.enter_context(tc.tile_pool(name="psA", bufs=2, space="PSUM"))
        psD = ctx.enter_context(tc.tile_pool(name="psD", bufs=1, space="PSUM"))
        psC = ctx.enter_context(tc.tile_pool(name="psC", bufs=1, space="PSUM"))

        # ---- constants ----
        Ibf = const.tile([P, P], BF16, tag="Ibf")
        make_identity(nc, Ibf)
        ones32 = const.tile([P, D], BF16, tag="ones32")
        nc.vector.memset(ones32, 1.0)
        ones1 = const.tile([1, P], BF16, tag="ones1")
        nc.vector.memset(ones1, 1.0)

        wsb = {}
        for w in ("q", "k", "v", "o"):
            wt = const.tile([P, 2, F], BF16, tag=f"w{w}", name=f"w{w}sb")
            nc.sync.dma_start(
                out=wt, in_=w_dram[w].rearrange("(c p) f -> p c f", p=P)
            )
            wsb[w] = wt
        nT = const.tile([P, 2, NN], BF16, tag="nT")
        nc.sync.dma_start(out=nT, in_=ndT_dram.rearrange("(c p) q -> p c q", p=P))

        bqc = const.tile([P, 2], F32, tag="bqc")
        nc.sync.dma_start(out=bqc, in_=b_dram["q"].rearrange("(c p) -> p c", p=P))
        bqs = const.tile([P, 2], F32, tag="bqs")
        nc.vector.tensor_scalar_mul(bqs, bqc, SCALE)
        bkc = const.tile([P, 2], F32, tag="bkc")
        nc.sync.dma_start(out=bkc, in_=b_dram["k"].rearrange("(c p) -> p c", p=P))
        brow = {}
        for w in ("v", "o"):
            bf = const.tile([1, F], F32, tag=f"b{w}f", name=f"b{w}f")
            nc.sync.dma_start(out=bf, in_=b_dram[w][None, :])
            bh = const.tile([1, F], BF16, tag=f"b{w}h", name=f"b{w}h")
            nc.vector.tensor_copy(bh, bf)
            brow[w] = bh

        # ---- prologue: Q/K/V projections ----
        # qt/kt[p, hg, q]: head hg*4+j lives at partitions 32j..32j+31
        qt = qkvp.tile([P, 2, NN], BF16, tag="qt")
        kt = qkvp.tile([P, 2, NN], BF16, tag="kt")
        for name, dst in (("q", qt), ("k", kt)):
            for co in range(2):
                ps = psA.tile([P, NN], F32, tag="A", name=f"ps_{name}{co}")
                for s in range(2):
                    sl = slice(s * 512, (s + 1) * 512)
                    for ci in range(2):
                        nc.tensor.matmul(
                            ps[:, sl],
                            lhsT=wsb[name][:, ci, co * P : (co + 1) * P],
                            rhs=nT[:, ci, sl],
                            start=(ci == 0),
                            stop=(ci == 1),
                        )
                if name == "q":
                    nc.scalar.activation(
                        dst[:, co, :], ps, AF.Identity,
                        bias=bqs[:, co : co + 1], scale=SCALE,
                    )
                else:
                    nc.scalar.activation(
                        dst[:, co, :], ps, AF.Identity, bias=bkc[:, co : co + 1]
                    )

        # vp[p, kc, (h d)]: V rows for k-chunk kc
        vp = qkvp.tile([P, KC, F], BF16, tag="vp")
        for t in range(KC):
            psv = psA.tile([P, F], F32, tag="A", name=f"psv{t}")
            for ci in range(2):
                nc.tensor.matmul(
                    psv,
                    lhsT=nT[:, ci, t * P : (t + 1) * P],
                    rhs=wsb["v"][:, ci, :],
                    start=(ci == 0),
                    stop=False,
                )
            nc.tensor.matmul(psv, lhsT=ones1, rhs=brow["v"], start=False, stop=True)
            nc.scalar.copy(vp[:, t, :], psv)

        # ---- load phase ----
        # The bias streams in per-pass halves ([k, 0:4, q] then [k, 4:8, q]):
        # same total HBM traffic, but pass0 starts after ~1MB instead of
        # waiting out a 17MB resident load, and SBUF holds at most a few
        # half-chunks. The DMA FIFO order (pass0 halves, then pass1 halves)
        # makes pass1's bias arrive during pass0's compute.
        def bias_half_dma(hg, kc, first=False):
            bt = bias_pools[hg].tile(
                [P, 4 * NN], BF16, tag=f"bias{hg}", name=f"biasT_{hg}_{kc}"
            )
            nc.sync.dma_start(
                out=bt,
                in_=biasT_dram[
                    kc * P : (kc + 1) * P, 4 * hg : 4 * (hg + 1)
                ].rearrange("k h q -> k (h q)"),
            )
            bias_t[hg].append(bt)

        bias_pools = {0: biasp, 1: biasp1}
        bias_t = {0: [], 1: []}
        bias_half_dma(0, 0)
        m8 = []
        for kc in range(KC):
            m = mpool.tile([P, NN], U8, tag="m8", name=f"m8_{kc}")
            nc.sync.dma_start(out=m, in_=maskT_dram[kc * P : (kc + 1) * P, :])
            m8.append(m)
        for kc in range(1, KC):
            bias_half_dma(0, kc)
        for kc in range(KC):
            bias_half_dma(1, kc)
        # mask -> -1e30 bf16 (ACT), then fold into each head's bias slice
        # with one broadcast DVE add per half-chunk. Emission order matters:
        # each engine queue is FIFO, so pass0's folds interleave into the
        # pass0 loop (keeping the first exps near the queue head) and
        # pass1's folds are emitted between the passes (their DMAs land
        # mid-pass0; queueing them earlier would stall pass0's DVE adds).
        # chunk 0 partition 0 is the always-unmasked k=0 row: zero its mask.
        mnegs = {}

        def fold(hg, kc):
            if kc not in mnegs:
                mneg = mpool.tile(
                    [P, NN], BF16, tag="mneg", name=f"mneg_{kc}", bufs=8
                )
                nc.scalar.mul(mneg, m8[kc], NEG)
                if kc == 0:
                    nc.gpsimd.memset(mneg[0:1, :], 0.0)
                mnegs[kc] = mneg
            bv = bias_t[hg][kc].rearrange("k (h q) -> k h q", q=NN)
            bvb, mbb = bass.broadcast_tensor_aps(bv, mnegs[kc][:, None, :])
            nc.vector.tensor_add(bvb, bvb, mbb)

        fold(0, 0)
        fold(0, 1)

        # ---- two passes of 4 heads each ----
        outT = []
        for hg in range(2):
            if hg == 1:
                for kc in range(KC):
                    fold(1, kc)
            den = psD.tile([P, NN], F32, tag="D", name=f"den{hg}")
            psc = psC.tile([P, NN], F32, tag="C", name=f"psc{hg}")

            def emit_backs(kc, etiles):
                # kind-grouped across the 4 col bands: consecutive same-kind
                # M=32 matmuls to distinct 32-col array bands can pack, and
                # the den matmuls share the ones32 stationary operand.
                # start=True per band at kc==0 clears/overwrites only that
                # matmul's own PSUM footprint (per-element has_written).
                for s in range(2):
                    sl = slice(s * 512, (s + 1) * 512)
                    for j in range(4):
                        nc.tensor.matmul(
                            den[32 * j : 32 * (j + 1), sl],
                            lhsT=ones32,
                            rhs=etiles[j][:, sl],
                            start=(kc == 0),
                            stop=(kc == KC - 1),
                            tile_position=(0, 32 * j),
                            skip_group_check=True,
                        )
                for s in range(2):
                    sl = slice(s * 512, (s + 1) * 512)
                    for j in range(4):
                        h = hg * 4 + j
                        nc.tensor.matmul(
                            psc[32 * j : 32 * (j + 1), sl],
                            lhsT=vp[:, kc, h * D : (h + 1) * D],
                            rhs=etiles[j][:, sl],
                            start=(kc == 0),
                            stop=(kc == KC - 1),
                            tile_position=(0, 32 * j),
                            skip_group_check=True,
                        )

            prev = None
            for kc in range(KC):
                if hg == 0 and kc + 2 < KC:
                    fold(0, kc + 2)
                cur = []
                for j in range(4):
                    h = hg * 4 + j
                    psa = psA.tile([P, NN], F32, tag="A", name=f"psa_{hg}_{kc}_{j}")
                    # both halves of QK share one kt LDWEIGHTS; on-PE bias
                    # adds share one Ibf LDWEIGHTS
                    on_pe = hg == 0 and j < 2
                    for s in range(2):
                        sl = slice(s * 512, (s + 1) * 512)
                        nc.tensor.matmul(
                            psa[:, sl],
                            lhsT=kt[32 * j : 32 * (j + 1), hg, kc * P : (kc + 1) * P],
                            rhs=qt[32 * j : 32 * (j + 1), hg, sl],
                            start=True,
                            stop=not on_pe,
                            tile_position=(32 * j, 0),
                            skip_group_check=True,
                        )
                    e = epool.tile([P, NN], BF16, tag="e", name=f"e_{hg}_{kc}_{j}")
                    if on_pe:
                        # bias add on PE while the load phase gates the pace
                        for s in range(2):
                            sl = slice(s * 512, (s + 1) * 512)
                            nc.tensor.matmul(
                                psa[:, sl],
                                lhsT=Ibf,
                                rhs=bias_t[0][kc][
                                    :, j * NN + s * 512 : j * NN + (s + 1) * 512
                                ],
                                start=False,
                                stop=True,
                                skip_group_check=True,
                            )
                        nc.scalar.activation(e, psa, AF.Exp)
                    else:
                        # bias add on DVE
                        sP = spool.tile([P, NN], BF16, tag="sP", name=f"sP_{hg}_{kc}_{j}")
                        nc.vector.tensor_add(
                            sP, psa, bias_t[hg][kc][:, j * NN : (j + 1) * NN]
                        )
                        nc.scalar.activation(e, sP, AF.Exp)
                    cur.append(e)
                if prev is not None:
                    emit_backs(kc - 1, prev)
                prev = cur
            emit_backs(KC - 1, prev)

            rec = yp.tile([P, NN], F32, tag="rec", name=f"rec{hg}", bufs=1)
            if USE_RECIP_APPROX:
                nc.vector.reciprocal_approx_fast(rec, den)
            else:
                nc.vector.reciprocal(rec, den)
            oT = otp.tile([P, NN], BF16, tag="oT", name=f"outT{hg}")
            nc.vector.tensor_mul(oT, psc, rec)
            outT.append(oT)

        # ---- output projection (psy spread over all psum pools so the
        # eight t-chunks overlap instead of serializing on two slots) ----
        ypool = [(psA, "A"), (psA, "A"), (psD, "D"), (psC, "C")]
        for t in range(KC):
            pool, ptag = ypool[t % 4]
            psy = pool.tile([P, F], F32, tag=ptag, name=f"psy{t}")
            for hg in range(2):
                nc.tensor.matmul(
                    psy,
                    lhsT=outT[hg][:, t * P : (t + 1) * P],
                    rhs=wsb["o"][:, hg, :],
                    start=(hg == 0),
                    stop=False,
                )
            nc.tensor.matmul(psy, lhsT=ones1, rhs=brow["o"], start=False, stop=True)
            y = yp.tile([P, F], BF16, tag="y", name=f"y{t}")
            nc.scalar.copy(y, psy)
            nc.sync.dma_start(out=out_dram[t * P : (t + 1) * P, :], in_=y)


_CACHE = {}


def _make_in_maps(inputs):
    import ml_dtypes

    bf16 = ml_dtypes.bfloat16
    nd = np.asarray(inputs["ndata"], np.float32)
    ab = np.asarray(inputs["attn_bias"], np.float32).astype(bf16)
    am = np.asarray(inputs["attn_mask"]).astype(np.uint8)
    ws = {
        f"wT{w}": np.ascontiguousarray(
            np.asarray(inputs[f"W{w}"], np.float32).T
        ).astype(bf16)
        for w in ("q", "k", "v", "o")
    }
    bs = {
        f"b{w}": np.asarray(inputs[f"b{w}"], np.float32) for w in ("q", "k", "v", "o")
    }
    in_maps = []
    for b in range(nd.shape[0]):
        m = {
            "ndT": np.ascontiguousarray(nd[b].T).astype(bf16),
            "biasT": np.ascontiguousarray(ab[b].transpose(1, 2, 0)),
            "maskT": np.ascontiguousarray(am[b].T),
        }
        m.update(ws)
        m.update(bs)
        in_maps.append(m)
    return in_maps


def _get_nc():
    if "nc" not in _CACHE:
        _CACHE["nc"] = build_program()
    return _CACHE["nc"]


def _ensure_ntff_hook():
    """Shim antenv.axon_hooks (absent in this image) so trace=True works."""
    import types

    try:
        from antenv.axon_hooks import get_axon_ntff_profile_hook  # noqa: F401

        return
    except ImportError:
        pass
    import antenv

    mod = types.ModuleType("antenv.axon_hooks")
    _h = [None]
    mod.set_axon_ntff_profile_hook = lambda h: _h.__setitem__(0, h)
    mod.get_axon_ntff_profile_hook = lambda: _h[0]
    sys.modules["antenv.axon_hooks"] = mod
    antenv.axon_hooks = mod
    from trn_agent_boot.trn_boot import _ntff_profile_via_ctypes

    mod.set_axon_ntff_profile_hook(
        _ntff_profile_via_ctypes("/opt/axon/libaxon_pjrt.so")
    )


def run(inputs, trace=False):
    """Run on hardware; returns (output (B,N,F) f32, exec_time_ns or None)."""
    from concourse import bass_utils

    if trace:
        _ensure_ntff_hook()
    nc = _get_nc()
    in_maps = _make_in_maps(inputs)
    res = bass_utils.run_bass_kernel_spmd(
        nc, in_maps, core_ids=list(range(len(in_maps))), trace=trace
    )
    out = np.stack([r["out"] for r in res.results]).astype(np.float32)
    return out, res.exec_time_ns


def kernel(**inputs):
    out, _ = run(inputs, trace=False)
    return out
